# revision 56
# baseline (speedup 1.0000x reference)
"""ComplexAttentionLayer Trainium2 kernel, v3 (8-core data-parallel).

Math (per token t, head h; E=64; per-head feature dim is 1, so scores are
outer products over the E axis):
  w[l,s]   = Gp[l]*Hm[s] + Gm[l]*Hp[s]       (= 2*abs2, PE outer products)
             Gp=(qr+qi)^2, Gm=(qr-qi)^2, Hp=(kr+ki)^2, Hm=(kr-ki)^2
  score    = sqrt(0.5*w)                      (ACT Sqrt table, exact)
  E        = exp(score)   via the Schraudolph bf16 bit trick on the DVE:
             bits16 = round(A2*score + 16256), A2 = 128/ln2; the bf16 with
             those bits is exp(score)*R(phi), R in [1, 1.0613] a mantissa
             sawtooth.  A second sample bits16+64 shifts the sawtooth phase
             by half a period (and multiplies by sqrt2); contracting
             E1 against V and E2 against V/sqrt2 in one accumulating PSUM
             group averages the two phases: residual error ~ +-0.8%.
  out[l]   = sum_s E[l,s] v[s] / sum_s E[l,s]  (PE per-token matmuls with a
             ones column for the denominator; DVE reciprocal+mul normalize)

The ACT engine runs ONLY the sqrt pass (one table set, loaded once); the
exp lives on the DVE at its 4x (2-byte) rate; abs2/contraction/projections
are PE matmuls; staging uses 8 flatten-DMAs per (tt, quarter) and V is
transposed with the XBAR dma_start_transpose.
"""

import math

import numpy as np

import concourse.bass as bass
import concourse.tile as tile
from concourse import bacc, mybir
from concourse.bass_utils import run_bass_kernel_spmd

AF = mybir.ActivationFunctionType
ALU = mybir.AluOpType
F32 = mybir.dt.float32
F16 = mybir.dt.float16
I16 = mybir.dt.int16
BF16 = mybir.dt.bfloat16

B, L, D, H = 4, 1024, 512, 8
E = D // H           # 64
NCORES = 8
T = B * L // NCORES  # 512 tokens per core
PT = 128             # tokens per tile
NTT = T // PT        # 4 token tiles per core
KT = D // 128        # 4 k-tiles per weight
NJQ = 4              # token quarters per tile
TQ = PT // NJQ       # 32 tokens per (tt, jq)
NPJ = TQ // 2        # 16 pairs per (tt, jq)

A2 = 128.0 / math.log(2.0)
SQ_SCALE = 0.5 * A2 * A2   # sqrt(SQ_SCALE*w) = A2*sqrt(0.5*w) = A2*score
BPRIME = 16256.0
INV_SQRT2 = 1.0 / math.sqrt(2.0)


def _build_module():
    nc = bacc.Bacc()

    xT = {}
    for nm in ("q_r", "q_i", "k_r", "k_i", "v_r", "v_i"):
        xT[nm] = nc.declare_dram_parameter(f"x_{nm}_T", [D, T], BF16, isOutput=False)
    w = {}
    for p in ("q", "k", "v", "o"):
        for c in ("r", "i", "in"):  # r = w_r.T, i = w_i.T, in = -w_i.T
            w[p, c] = nc.declare_dram_parameter(f"w_{p}_{c}", [D, D], BF16,
                                                isOutput=False)
    bias = {}
    for p in ("q", "k", "v", "o"):
        for c in ("r", "i"):  # r: br-bi, i: br+bi
            bias[p, c] = nc.declare_dram_parameter(f"b_{p}_{c}", [1, D], BF16,
                                                   isOutput=False)
    out_r = nc.declare_dram_parameter("out_r", [T, D], F32, isOutput=True)
    out_i = nc.declare_dram_parameter("out_i", [T, D], F32, isOutput=True)

    with tile.TileContext(nc) as tc:
        with (
            tc.tile_pool(name="const", bufs=1) as const_pool,
            tc.tile_pool(name="xin", bufs=1) as x_pool,
            tc.tile_pool(name="wgt", bufs=1) as w_pool,
            tc.tile_pool(name="stage", bufs=1) as stage_pool,
            tc.tile_pool(name="gstage", bufs=2) as gs_pool,
            tc.tile_pool(name="v2", bufs=1) as v2_pool,
            tc.tile_pool(name="v6", bufs=2) as v6_pool,
            tc.tile_pool(name="gh", bufs=2) as gh_pool,
            tc.tile_pool(name="evac", bufs=1) as evac_pool,
            tc.tile_pool(name="sco", bufs=3) as s_pool,
            tc.tile_pool(name="et", bufs=2) as et_pool,
            tc.tile_pool(name="nrm", bufs=1) as norm_pool,
            tc.tile_pool(name="ps", bufs=3, space="PSUM") as ps_pool,
            tc.tile_pool(name="psc", bufs=2, space="PSUM") as ctr_pool,
        ):
            ones_row = const_pool.tile([1, 128], BF16, tag="ones")
            nc.gpsimd.memset(ones_row[:], 1.0)

            ball = const_pool.tile([1, 8, D], BF16, tag="ball")
            bs = {}


            # o-projection operands (h-major d' = h*64+l)
            VrT = const_pool.tile([128, KT, T], BF16, tag="VrT")
            ViT = const_pool.tile([128, KT, T], BF16, tag="ViT")

            def load_kxn(pool, dram, tag, n, eng=None):
                t = pool.tile([128, KT, n], BF16, name=tag, tag=tag)
                (eng or nc.gpsimd).dma_start(
                    t[:], dram[:].rearrange("(k p) n -> p k n", p=128))
                return t

            def cproj(wset, xr_t, xi_t, tt):
                """complex linear on token tile tt -> psum [128, 1024]
                (yr cols 0:512, yi cols 512:1024)"""
                ts = slice(0, PT)
                ps = ps_pool.tile([128, NPJ, E], F32, tag="ps")
                flat = ps[:].rearrange("p a b -> p (a b)")
                yr = flat[:, 0:D]
                yi = flat[:, D:2 * D]
                for k in range(KT):
                    nc.tensor.matmul(yr, xr_t[:, k, ts], wset["r"][:, k, :],
                                     start=(k == 0), stop=False)
                for k in range(KT):
                    nc.tensor.matmul(yr, xi_t[:, k, ts], wset["in"][:, k, :],
                                     start=False, stop=False)
                nc.tensor.matmul(yr, ones_row[:], wset["br"],
                                 start=False, stop=True)
                for k in range(KT):
                    nc.tensor.matmul(yi, xi_t[:, k, ts], wset["r"][:, k, :],
                                     start=(k == 0), stop=False)
                for k in range(KT):
                    nc.tensor.matmul(yi, xr_t[:, k, ts], wset["i"][:, k, :],
                                     start=False, stop=False)
                nc.tensor.matmul(yi, ones_row[:], wset["bi"],
                                 start=False, stop=True)
                return ps, yr, yi

            def nat(ap):
                # [p, (l h)] natural projection cols -> [p, l, h]
                return ap.rearrange("p (l h) -> p l h", h=H)

            # ---- projections, software-pipelined per tt ----
            def load_proj(p):
                wt = p if p != "o" else "q"
                ws = {c: load_kxn(w_pool, w[p, c], f"w{wt}{c}", D,
                                  eng=nc.sync)
                      for c in ("r", "i", "in")}
                ws["br"] = bs[p, "r"]
                ws["bi"] = bs[p, "i"]
                return ws

            _border = [("q", "r"), ("q", "i"), ("k", "r"), ("k", "i"),
                       ("v", "r"), ("v", "i"), ("o", "r"), ("o", "i")]
            for _i, _key in enumerate(_border):
                bs[_key] = ball[:, _i, :]

            def load_biases(keys):
                for key in keys:
                    i = _border.index(key)
                    nc.sync.dma_start(ball[:, i, :], bias[key][:])

            def load_x(p, tt):
                xr_t = x_pool.tile([128, KT, PT], BF16, name=f"x{p}r",
                                   tag=f"x{p}r")
                xi_t = x_pool.tile([128, KT, PT], BF16, name=f"x{p}i",
                                   tag=f"x{p}i")
                sl = bass.ts(tt, PT)
                nc.gpsimd.dma_start(
                    xr_t[:],
                    xT[f"{p}_r"][:].rearrange("(k p) n -> p k n", p=128)[:, :, sl])
                nc.gpsimd.dma_start(
                    xi_t[:],
                    xT[f"{p}_i"][:].rearrange("(k p) n -> p k n", p=128)[:, :, sl])
                return xr_t, xi_t

            # PE p-state warmup: ~3us of junk matmuls while inputs load
            for _ in range(2):
                wps = ps_pool.tile([128, NPJ, E], F32, tag="ps")
                wf = wps[:].rearrange("p a b -> p (a b)")
                for i in range(8):
                    nc.tensor.matmul(wf[:, bass.ts(i, 128)], ones_row[:],
                                     ones_row[:])

            prj = {}
            for pp_ in ("q", "k", "v"):
                prj[pp_] = None  # placeholder, filled below in load order
            # q weights first (they gate the first projection), then biases,
            # then the rest
            ws_q = {c: load_kxn(w_pool, w["q", c], f"wq{c}", D, eng=nc.sync)
                    for c in ("r", "i", "in")}
            load_biases([("q", "r"), ("q", "i"), ("k", "r"), ("k", "i")])
            ws_q["br"] = bs["q", "r"]
            ws_q["bi"] = bs["q", "i"]
            prj["q"] = ws_q
            for pp_ in ("k", "v"):
                prj[pp_] = load_proj(pp_)
            load_biases([("v", "r"), ("v", "i"), ("o", "r"), ("o", "i")])
            v2_of = {}
            v6_of = {}
            gn_of = {}
            hn_of = {}

            def emit_qk(p, tt):
                # host-prepped weight combos make the projection emit
                # yr+yi (yr slot) and -(yr-yi) (yi slot) directly; one ACT
                # Square per half evacuates PSUM into Gn/Hn (sign drops)
                ws = prj[p]
                xr_t, xi_t = load_x(p, tt)
                ps, yr, yi = cproj(ws, xr_t, xi_t, tt)
                if p == "q":
                    dst = gh_pool.tile([PT, 2, H, E], BF16, name="Gn",
                                       tag="Gn")
                    gn_of[tt] = dst
                else:
                    dst = gh_pool.tile([PT, 2, H, E], BF16, name="Hn",
                                       tag="Hn")
                    hn_of[tt] = dst
                ty_p = 0 if p == "q" else 1
                ty_m = 1 - ty_p
                nc.scalar.activation(
                    dst[:, ty_p, :, :].rearrange("p h l -> p l h"),
                    nat(yr), AF.Square)
                nc.scalar.activation(
                    dst[:, ty_m, :, :].rearrange("p h l -> p l h"),
                    nat(yi), AF.Square)

            def emit_v(tt):
                ws = prj["v"]
                xr_t, xi_t = load_x("v", tt)
                ps, yr, yi = cproj(ws, xr_t, xi_t, tt)
                vfr = evac_pool.tile([128, H, 2, E], BF16, tag="vfr")
                vfi = evac_pool.tile([128, H, 2, E], BF16, tag="vfi")
                yr_h = yr.rearrange("p (h o e) -> p h o e", h=H, o=1)
                yi_h = yi.rearrange("p (h o e) -> p h o e", h=H, o=1)
                nc.scalar.copy(vfr[:], yr_h.broadcast_to([PT, H, 2, E]))
                nc.scalar.copy(vfi[:], yi_h.broadcast_to([PT, H, 2, E]))
                V2 = v2_pool.tile([128, H, 2, PT], BF16, tag="V2")
                nc.sync.dma_start_transpose(
                    V2[:, :, 0, :], vfr[:].rearrange("p a b c -> p (a b c)"))
                nc.sync.dma_start_transpose(
                    V2[:, :, 1, :], vfi[:].rearrange("p a b c -> p (a b c)"))
                v6 = v6_pool.tile([128, H, NJQ, NPJ, 6], BF16, tag="v6")
                v6b = v6_pool.tile([128, H, NJQ, NPJ, 6], BF16, tag="v6b")
                for c in range(2):
                    tv = V2[:, :, c, :].rearrange(
                        "p h (a blk b) -> p h a blk b", a=NJQ, blk=2)
                    nc.gpsimd.tensor_copy(v6[0:64, :, :, :, c],
                                          tv[0:64, :, :, 0, :])
                    nc.gpsimd.tensor_copy(v6[64:128, :, :, :, 3 + c],
                                          tv[64:128, :, :, 1, :])
                    nc.gpsimd.tensor_scalar(v6b[0:64, :, :, :, c],
                                            tv[0:64, :, :, 0, :],
                                            INV_SQRT2, None, op0=ALU.mult)
                    nc.gpsimd.tensor_scalar(v6b[64:128, :, :, :, 3 + c],
                                            tv[64:128, :, :, 1, :],
                                            INV_SQRT2, None, op0=ALU.mult)
                v2_of[tt] = V2
                v6_of[tt] = (v6, v6b)

            # pre-zero the staging HS buffer's zero-slots (A rows carry
            # data in blk 0 slots, B rows in blk 1; the complement stays 0)
            hs0 = stage_pool.tile([4, NPJ, H, 2, E], BF16, name="hs0",
                                  tag="HS")
            nc.vector.memset(hs0[:], 0.0)
            # preset v6 zero and ones slots on both rotating buffers
            for _ in range(2):
                for tg in ("v6", "v6b"):
                    one = 1.0 if tg == "v6" else INV_SQRT2
                    t6 = v6_pool.tile([128, H, NJQ, NPJ, 6], BF16,
                                      name=f"pre_{tg}", tag=tg)
                    nc.vector.memset(t6[0:64, :, :, :, 3:6], 0.0)
                    nc.vector.memset(t6[64:128, :, :, :, 0:3], 0.0)
                    nc.vector.memset(t6[0:64, :, :, :, 2], one)
                    nc.vector.memset(t6[64:128, :, :, :, 5], one)

            wo = {}

            def load_wo():
                for c_ in ("r", "i", "in"):
                    wo[c_] = load_kxn(w_pool, w["o", c_], f"w{c_}", D,
                                      eng=nc.gpsimd)

            def emit_oproj(tt):
                ts = bass.ts(tt, PT)
                ps = ps_pool.tile([128, NPJ, E], F32, tag="ps")
                flat = ps[:].rearrange("p a b -> p (a b)")
                our = flat[:, 0:D]
                oui = flat[:, D:2 * D]
                for k in range(KT):
                    nc.tensor.matmul(our, VrT[:, k, ts], wo["r"][:, k, :],
                                     start=(k == 0), stop=False)
                for k in range(KT):
                    nc.tensor.matmul(our, ViT[:, k, ts], wo["in"][:, k, :],
                                     start=False, stop=False)
                nc.tensor.matmul(our, ones_row[:], bs["o", "r"],
                                 start=False, stop=True)
                for k in range(KT):
                    nc.tensor.matmul(oui, ViT[:, k, ts], wo["r"][:, k, :],
                                     start=(k == 0), stop=False)
                for k in range(KT):
                    nc.tensor.matmul(oui, VrT[:, k, ts], wo["i"][:, k, :],
                                     start=False, stop=False)
                nc.tensor.matmul(oui, ones_row[:], bs["o", "i"],
                                 start=False, stop=True)
                sor = evac_pool.tile([PT, D], F32, tag="sor")
                soi = evac_pool.tile([PT, D], F32, tag="soi")
                nc.vector.tensor_copy(sor[:], our)
                nc.vector.tensor_copy(soi[:], oui)
                nc.sync.dma_start(out_r[ts, :], sor[:])
                nc.sync.dma_start(out_i[ts, :], soi[:])

            # ---- attention, with next-tt projections emitted ahead ----
            def emit_staging(stt, sjq):
                arng = slice(sjq * TQ, sjq * TQ + NPJ)
                brng = slice(sjq * TQ + NPJ, sjq * TQ + TQ)
                Gn = gn_of[stt]
                Hn = hn_of[stt]
                GS = gs_pool.tile([4, NPJ, H, E], BF16, name="GS", tag="GS")
                HS = stage_pool.tile([4, NPJ, H, 2, E], BF16, name="HS",
                                     tag="HS")
                rows = ((arng, 0), (arng, 1), (brng, 0), (brng, 1))
                for r, (rng, ty) in enumerate(rows):
                    nc.sync.dma_start(GS[r:r + 1, :, :, :],
                                      Gn[rng, ty, :, :])
                # HS rows 0,1 via HWDGE; rows 2,3 deferred to Pool SWDGE
                # (emitted at iteration end) so the two DGE paths overlap
                for r, (rng, ty) in list(enumerate(rows))[:2]:
                    nc.sync.dma_start(HS[r:r + 1, :, :, r // 2, :],
                                      Hn[rng, ty, :, :])
                pool_fill[(stt, sjq)] = (HS, Hn, list(enumerate(rows))[2:])
                st_of[(stt, sjq)] = (GS, HS)

            def emit_staging_pool(stt, sjq):
                ent = pool_fill.pop((stt, sjq), None)
                if ent is None:
                    return
                HS, Hn, rows = ent
                for r, (rng, ty) in rows:
                    nc.gpsimd.dma_start(HS[r:r + 1, :, :, r // 2, :],
                                        Hn[rng, ty, :, :])

            st_of = {}
            pool_fill = {}
            emit_qk("q", 0)
            emit_qk("k", 0)
            emit_staging(0, 0)
            emit_staging(0, 1)
            emit_staging_pool(0, 0)
            emit_staging_pool(0, 1)
            emit_v(0)
            for tt in range(NTT):
                V2 = v2_of.pop(tt)
                v6, v6b = v6_of.pop(tt)
                for jq in range(NJQ):
                    # issue next quarter's staging ahead of everything else
                    njq = (jq + 1) % NJQ
                    ntt = tt + (1 if njq == 0 else 0)
                    if ntt < NTT and (tt + 1 < NTT or njq != 0):
                        if (ntt, njq) == (tt + 1, 0):
                            pass  # deferred below until Gn/Hn exist
                        elif (ntt, njq) not in st_of:
                            emit_staging(ntt, njq)
                    if tt == 1 and jq == 1:
                        load_wo()
                    if tt + 1 < NTT:
                        if jq == 1:
                            emit_qk("q", tt + 1)
                        elif jq == 2:
                            emit_qk("k", tt + 1)
                            emit_staging(tt + 1, 0)
                        elif jq == 3:
                            emit_v(tt + 1)
                            if tt >= 1:
                                emit_oproj(tt - 1)
                    elif jq == 1:
                        emit_oproj(tt - 1)
                    GS, HS = st_of.pop((tt, jq))

                    ctr = ctr_pool.tile([128, KT, TQ, 3], F32, tag="ctr")
                    S = None
                    for h in range(H):
                        par = h % 2
                        k = h // 2
                        ab = ps_pool.tile([128, NPJ, E], F32, tag="ps")
                        for j in range(NPJ):
                            nc.tensor.matmul(ab[:, j, :],
                                             HS[0:4, j, h, :, :],
                                             GS[0:4, j, h, :])
                        if par == 0:
                            S = s_pool.tile([128, 2, NPJ, E], F16, tag="S")
                        nc.scalar.activation(
                            S[:, par, :, :].rearrange("p a b -> p (a b)"),
                            ab[:].rearrange("p a b -> p (a b)"),
                            AF.Sqrt, scale=SQ_SCALE)
                        if par == 0:
                            continue
                        Et1 = et_pool.tile([128, 2, NPJ, E], BF16, tag="E1")
                        Et2 = et_pool.tile([128, 2, NPJ, E], BF16, tag="E2")
                        sflat = S[:].rearrange("p a b c -> p (a b c)")
                        nc.vector.tensor_scalar(
                            Et1[:].rearrange("p a b c -> p (a b c)").bitcast(I16),
                            sflat, BPRIME, None, op0=ALU.add)
                        nc.vector.tensor_scalar(
                            Et2[:].rearrange("p a b c -> p (a b c)").bitcast(I16),
                            Et1[:].rearrange("p a b c -> p (a b c)").bitcast(I16),
                            64.0, None, op0=ALU.add)
                        for hh in (h - 1, h):
                            pp = hh % 2
                            base = 64 * pp
                            cv = ctr[base:base + 64, k, :, :].rearrange(
                                "p (blk jl) c -> p jl blk c", blk=2)
                            for j in range(NPJ):
                                dst = cv[:, j, :, :]
                                nc.tensor.matmul(
                                    dst, Et1[:, pp, j, :],
                                    v6[:, hh, jq, j, :],
                                    start=True, stop=False)
                                nc.tensor.matmul(
                                    dst, Et2[:, pp, j, :],
                                    v6b[:, hh, jq, j, :],
                                    start=False, stop=True)

                    # normalize + write o-proj operands
                    rcp = norm_pool.tile([128, KT, TQ], F32, tag="rcp")
                    nc.vector.reciprocal(rcp[:], ctr[:, :, :, 2])
                    tsl = slice(tt * PT + jq * TQ, tt * PT + (jq + 1) * TQ)
                    nc.vector.tensor_mul(VrT[:, :, tsl], ctr[:, :, :, 0],
                                         rcp[:])
                    nc.vector.tensor_mul(ViT[:, :, tsl], ctr[:, :, :, 1],
                                         rcp[:])
                    njq2 = (jq + 1) % NJQ
                    ntt2 = tt + (1 if njq2 == 0 else 0)
                    emit_staging_pool(ntt2, njq2)

            # ---- phase 4: final output projection ----
            emit_oproj(NTT - 1)

    nc.compile()
    return nc


_NC_CACHE = None


def _get_module():
    global _NC_CACHE
    if _NC_CACHE is None:
        _NC_CACHE = _build_module()
    return _NC_CACHE


def _prep_inputs(inputs):
    """host-side shard/layout prep -> list of 8 per-core input maps"""
    import ml_dtypes
    bf = ml_dtypes.bfloat16
    TT = B * L
    xs = {nm: np.ascontiguousarray(
        np.asarray(inputs[nm]).reshape(TT, D).T.astype(bf))
        for nm in ("q_r", "q_i", "k_r", "k_i", "v_r", "v_i")}
    # h-major permutation d' = h*64 + l  ->  natural col l*H + h
    perm = np.empty(D, np.int64)
    for h in range(H):
        for l in range(E):
            perm[h * E + l] = l * H + h
    common = {}
    for p in ("q", "k", "v", "o"):
        wr = np.asarray(inputs[f"w{p}_r"]).astype(np.float32)
        wi = np.asarray(inputs[f"w{p}_i"]).astype(np.float32)
        br = np.asarray(inputs[f"b{p}_r"]).astype(np.float32)
        bi = np.asarray(inputs[f"b{p}_i"]).astype(np.float32)
        wrT = wr.T
        wiT = wi.T
        bm = br - bi
        bp = br + bi
        if p in ("q", "k"):
            # projection emits tp = yr+yi and -(yr-yi):
            #   tp = xr@(wr+wi).T + xi@(wr-wi).T + 2 br
            #   tm'= xi@(wr+wi).T - xr@(wr-wi).T + 2 bi   (= -(yr-yi))
            wsum = wrT + wiT
            wdif = wrT - wiT
            common[f"w_{p}_r"] = np.ascontiguousarray(wsum.astype(bf))
            common[f"w_{p}_in"] = np.ascontiguousarray(wdif.astype(bf))
            common[f"w_{p}_i"] = np.ascontiguousarray((-wdif).astype(bf))
            common[f"b_{p}_r"] = (2 * br).reshape(1, D).astype(bf)
            common[f"b_{p}_i"] = (2 * bi).reshape(1, D).astype(bf)
            continue
        if p == "o":
            # o-proj contracts over h-major d': permute weight rows
            wrT = wrT[perm, :]
            wiT = wiT[perm, :]
        if p == "v":
            # v-proj emits h-major cols: permute weight cols + bias
            wrT = wrT[:, perm]
            wiT = wiT[:, perm]
            bm = bm[perm]
            bp = bp[perm]
        common[f"w_{p}_r"] = np.ascontiguousarray(wrT.astype(bf))
        common[f"w_{p}_i"] = np.ascontiguousarray(wiT.astype(bf))
        common[f"w_{p}_in"] = np.ascontiguousarray((-wiT).astype(bf))
        common[f"b_{p}_r"] = bm.reshape(1, D).astype(bf)
        common[f"b_{p}_i"] = bp.reshape(1, D).astype(bf)
    maps = []
    for c in range(NCORES):
        m = dict(common)
        sl = slice(c * T, (c + 1) * T)
        for nm, arr in xs.items():
            m[f"x_{nm}_T"] = np.ascontiguousarray(arr[:, sl])
        maps.append(m)
    return maps


def kernel(**inputs):
    nc = _get_module()
    maps = _prep_inputs(inputs)
    res = run_bass_kernel_spmd(nc, maps, core_ids=list(range(NCORES)))
    out_r = np.concatenate([res.results[c]["out_r"] for c in range(NCORES)],
                           axis=0).reshape(B, L, D)
    out_i = np.concatenate([res.results[c]["out_i"] for c in range(NCORES)],
                           axis=0).reshape(B, L, D)
    return out_r, out_i
